# revision 25
# baseline (speedup 1.0000x reference)
"""Multi-head attention (B=4, N=2048, C=256, H=8) on 8 Trainium2 NeuronCores.

Sharding: core c handles batch b = c//2 and query-half qh = c%2 (1024 query
rows), all 8 heads. k/v are computed for the full sequence on each core (the
qkv projection is cheap); outputs concatenate with no cross-core reduction.

Device-side layout is fully "transposed" (channels on partitions):
  - x^T [C, N] feeds q^T/k^T ([d, tokens], head-major rows) and v ([tokens, d]).
  - Scores are computed as S^T [k-tokens, q-tokens] so that softmax's exp
    output E^T feeds the AV matmul directly (contraction over k on partitions).
  - Softmax denominators come for free as a 33rd "ones" column appended to v.
  - O^T [channels, q] feeds the output projection directly.
Softmax skips max-subtraction: scaled scores are ~N(0,1) (max |s| < ~10), safe
in fp32 exp. Matmuls run in float32r (full PE rate at free dim >= 256); every
float32r operand is produced by a rounding-capable instruction (DVE copy, ACT).

The emission is software-pipelined so TensorE (the busiest engine, ~123us of
matmul streaming) never stalls:
  - exp reads each [128, 1024] score chunk straight from PSUM on ScalarE
    (no DVE staging copy); the last POOL_COLS columns of each chunk are
    exp'd on the otherwise-idle GPSIMD engine (pow) from a small pre-scaled
    DVE-staged SBUF copy, keeping ScalarE below the PE pace.
  - AV matmuls trail their chunk by one tick to hide the exp latency.
  - Most of the qkv projection is spread across the first iterations' chunk
    loop (worklist), so exp starts after a minimal prefix.
  - Normalization and the output projection of a finished q-block are emitted
    a few chunks into the following iteration.
"""

import os
from contextlib import ExitStack

import numpy as np

import concourse.bacc as bacc
import concourse.bass as bass
import concourse.mybir as mybir
import concourse.tile as tile
from concourse import library_config
from concourse.bass_utils import run_bass_kernel_spmd

B, N, C = 4, 2048, 256
H, D = 8, 32
P = 128
QH = N // 2              # query rows per core
SCALE = float(D) ** -0.5
NCORES = 8
NCH = N // P             # 16 k-chunks

F32 = mybir.dt.float32
F32R = mybir.dt.float32r
EXP = mybir.ActivationFunctionType.Exp

# Timing amplification for the local harness (repeat attention+proj body).
REPS = int(os.environ.get("BASS_ATTN_REPS", "1"))
REPS_MODE = os.environ.get("BASS_ATTN_REPS_MODE", "loop")  # "loop" | "unroll"
# Columns of each [128, 1024] score chunk exp'd on GPSIMD (pow) instead of
# ScalarE; must be a multiple of 128 (0 disables the offload).
POOL_COLS = int(os.environ.get("BASS_ATTN_POOL_COLS", "256"))
ACT_COLS = 1024 - POOL_COLS
E_CONST = 2.718281828459045


def _emit(tc, xT, xTq, wall, pb, y):
    nc = tc.nc
    with ExitStack() as ctx:
        singles = ctx.enter_context(tc.tile_pool(name="singles", bufs=1))
        epool = ctx.enter_context(tc.tile_pool(name="epool", bufs=5))
        spool = ctx.enter_context(tc.tile_pool(name="spool", bufs=4))
        small = ctx.enter_context(tc.tile_pool(name="small", bufs=4))
        ypool = ctx.enter_context(tc.tile_pool(name="ypool", bufs=3))
        # PSUM budget (8 banks): s-ring 2x2 + po 3x1 + scratch 1x1
        ps = ctx.enter_context(tc.tile_pool(name="ps", bufs=2, space="PSUM"))
        po = ctx.enter_context(tc.tile_pool(name="po", bufs=3, space="PSUM"))

        # ---- input loads + fp32r rounding (staged) -----------------------
        def load_r(name, dram_ap, cshape):
            ld = singles.tile(cshape, F32, tag=name + "_ld", name=name + "_ld")
            nc.sync.dma_start(ld[:], dram_ap)
            rt = singles.tile(cshape, F32R, tag=name, name=name)
            nc.vector.tensor_copy(rt[:], ld[:])
            return rt

        # all four weight matrices arrive packed in one dram tensor (one DMA,
        # one rounding copy); order: wk, wq, wv, pw
        wall_ld = singles.tile([P, 2, 4 * C], F32, tag="wall_ld", name="wall_ld")
        wall_sb = singles.tile([P, 2, 4 * C], F32R, tag="wall", name="wall_sb")
        wall4 = wall_sb[:].rearrange("p c (w n) -> p c w n", n=C)
        wk_sb = wall4[:, :, 0]
        wq_sb = wall4[:, :, 1]
        wv_sb = wall4[:, :, 2]
        pw_sb = wall4[:, :, 3]
        # x^T loads staged + rounded in 512-column blocks so the first qkv
        # matmuls (and the attention stream behind them) start early.
        xT_ld = singles.tile([P, 2, N], F32, tag="xT_ld", name="xT_ld")
        xT_sb = singles.tile([P, 2, N], F32R, tag="xT", name="xT_sb")
        xT_r = xT.rearrange("(c p) n -> p c n", p=P)

        def load_x_block(nb):
            sl = (slice(None), slice(None), slice(512 * nb, 512 * nb + 512))
            nc.sync.dma_start(xT_ld[sl], xT_r[sl])
            nc.vector.tensor_copy(xT_sb[sl], xT_ld[sl])

        load_x_block(0)
        nc.sync.dma_start(wall_ld[:], wall.rearrange("(c p) n -> p c n", p=P))
        # round the k-projection slice first: it gates the first matmuls
        nc.vector.tensor_copy(wall_sb[:, :, 0:C], wall_ld[:, :, 0:C])
        nc.vector.tensor_copy(wall_sb[:, :, C : 4 * C], wall_ld[:, :, C : 4 * C])
        xTq_ld = singles.tile([P, 2, QH], F32, tag="xTq_ld", name="xTq_ld")
        xTq_sb = singles.tile([P, 2, QH], F32R, tag="xTq", name="xTq_sb")
        xTq_r = xTq.rearrange("(c p) n -> p c n", p=P)

        def load_xq_block(nb):
            sl = (slice(None), slice(None), slice(512 * nb, 512 * nb + 512))
            nc.sync.dma_start(xTq_ld[sl], xTq_r[sl])
            nc.vector.tensor_copy(xTq_sb[sl], xTq_ld[sl])

        load_xq_block(0)
        for nb in range(1, 4):
            load_x_block(nb)
        load_xq_block(1)
        pb_sb = singles.tile([P, C], F32, tag="pb")
        nc.sync.dma_start(
            pb_sb[:],
            bass.AP(tensor=pb.tensor, offset=pb.offset, ap=[[0, P]] + list(pb.ap)),
        )

        # ---- qkv projection emitters ------------------------------------
        # q^T/k^T stacks: chunk cc holds heads 4cc..4cc+3 at rows 32*(h%4).
        qT_sb = singles.tile([P, 2, QH], F32R, tag="qT")
        kT_sb = singles.tile([P, 2, N], F32R, tag="kT")
        # v_aug: [token-tile, head-major (v_h | 1)] for AV + denominator.
        # In the packed po tile the e0 den lands at row 32 and the e1 den at
        # row 96 -- both 32-aligned, as engine partition access requires.
        vA_sb = singles.tile([P, NCH, H * (D + 1)], F32R, tag="vA")
        onesF = singles.tile([P, NCH, H], F32, tag="onesF")
        nc.vector.memset(onesF[:], 1.0)
        vA4 = vA_sb[:].rearrange("p t (h a) -> p t h a", a=D + 1)
        nc.vector.tensor_copy(vA4[:, :, :, D], onesF[:])

        def emit_kqT(w_sb, x_sb, out_sb, cc, nb):
            pk = ps.tile([P, 512], F32, tag="bc", bufs=1, name="pk")
            for ci in range(2):
                nc.tensor.matmul(
                    pk[:],
                    lhsT=w_sb[:, ci, 128 * cc : 128 * cc + 128],
                    rhs=x_sb[:, ci, 512 * nb : 512 * nb + 512],
                    start=(ci == 0),
                    stop=(ci == 1),
                )
            # ACT Copy (shares the exp table set): DVE stays free for staging
            nc.scalar.copy(out_sb[:, cc, 512 * nb : 512 * nb + 512], pk[:])

        def emit_v(tt):
            pv = ps.tile([P, 512], F32, tag="bc", bufs=1, name="pv")
            for ci in range(2):
                nc.tensor.matmul(
                    pv[:, 0:256],
                    lhsT=xT_sb[:, ci, 128 * tt : 128 * tt + 128],
                    rhs=wv_sb[:, ci, :],
                    start=(ci == 0),
                    stop=(ci == 1),
                )
            nc.scalar.copy(
                vA4[:, tt, :, 0:D],
                pv[:, 0:256].rearrange("p (h d) -> p h d", d=D),
            )

        # ---- attention helpers ------------------------------------------
        ones_f2 = singles.tile([1, 32], F32, tag="onesf2")
        nc.vector.memset(ones_f2[:], 1.0)
        ones_sb = singles.tile([1, 32], F32R, tag="ones")
        nc.vector.tensor_copy(ones_sb[:], ones_f2[:])
        econst = None
        if POOL_COLS:
            nc.gpsimd.load_library(library_config.mlp)
            econst = singles.tile([P, POOL_COLS], F32, tag="econst")
            nc.vector.memset(econst[:], E_CONST)
        OT_sb = singles.tile([P, 2, QH], F32R, tag="OT")

        def emit_proj_qt(qb, qt):
            tq = 4 * qb + qt
            py = ps.tile([P, 512], F32, tag="bc", bufs=1, name="py")
            for ci in range(2):
                nc.tensor.matmul(
                    py[:, 0:256],
                    lhsT=OT_sb[:, ci, 128 * tq : 128 * tq + 128],
                    rhs=pw_sb[:, ci, :],
                    start=(ci == 0),
                    stop=(ci == 1),
                )
            ysb = ypool.tile([P, C], F32, tag="y", name="ysb")
            nc.vector.tensor_add(ysb[:], py[:, 0:256], pb_sb[:])
            nc.sync.dma_start(y[128 * tq : 128 * tq + 128, :], ysb[:])

        # normalization of one head, split into three one-per-tick stages so
        # the DVE never does more than ~one extra op per chunk tick and the PE
        # has no dependency on the chain (broadcast runs on GPSIMD).
        def norm_rcp(pots, e):
            rcp = small.tile([1, 512], F32R, tag="rcp", name="rcp")
            with nc.allow_low_precision(reason="1/den rounds to f32r as before"):
                nc.vector.reciprocal(rcp[:], pots[e][D : D + 1, :])
            return rcp

        def norm_bcast(rcp):
            bc = small.tile([32, 512], F32R, tag="bcs", name="bcs")
            nc.gpsimd.partition_broadcast(bc[:], rcp[:])
            return bc

        def norm_mul(bc, pots, hp, qb, e):
            h = 2 * hp + e
            r, cc = 32 * (h % 4), h // 4
            nc.vector.tensor_mul(
                OT_sb[r : r + 32, cc, 512 * qb : 512 * qb + 512],
                pots[e][0:D, :],
                bc[:],
            )

        def emit_av(pots, hp, et, ch):
            # et: [P, 1024] AP (heads side by side) for chunk ch
            for e in range(2):
                h = 2 * hp + e
                nc.tensor.matmul(
                    pots[e][:],
                    lhsT=vA_sb[:, ch, (D + 1) * h : (D + 1) * (h + 1)],
                    rhs=et[:, 512 * e : 512 * e + 512],
                    start=(ch == 0),
                    stop=(ch == NCH - 1),
                    skip_group_check=True,
                )

        def emit_body():
            # ---- prefix: just enough qkv for iteration 0's first chunks ------
            emit_kqT(wk_sb, xT_sb, kT_sb, 0, 0)
            emit_kqT(wq_sb, xTq_sb, qT_sb, 0, 0)
            for tt in range(4):
                emit_v(tt)
            # remaining qkv work, spread one item per chunk tick; deadlines:
            # v_tt by tick tt, kT(0,nb) before tick 4nb, cc=1 before tick 32.
            worklist = [
                lambda: emit_v(4),
                lambda: emit_kqT(wk_sb, xT_sb, kT_sb, 0, 1),
                lambda: emit_v(5),
                lambda: emit_v(6),
                lambda: emit_v(7),
                lambda: emit_kqT(wk_sb, xT_sb, kT_sb, 0, 2),
                lambda: emit_v(8),
                lambda: emit_v(9),
                lambda: emit_v(10),
                lambda: emit_kqT(wk_sb, xT_sb, kT_sb, 0, 3),
                lambda: emit_v(11),
                lambda: emit_v(12),
                lambda: emit_v(13),
                lambda: emit_v(14),
                lambda: emit_v(15),
                lambda: emit_kqT(wq_sb, xTq_sb, qT_sb, 0, 1),
                lambda: emit_kqT(wk_sb, xT_sb, kT_sb, 1, 0),
                lambda: emit_kqT(wk_sb, xT_sb, kT_sb, 1, 1),
                lambda: emit_kqT(wk_sb, xT_sb, kT_sb, 1, 2),
                lambda: emit_kqT(wk_sb, xT_sb, kT_sb, 1, 3),
                lambda: emit_kqT(wq_sb, xTq_sb, qT_sb, 1, 0),
                lambda: emit_kqT(wq_sb, xTq_sb, qT_sb, 1, 1),
            ]

            # ---- attention main loop -----------------------------------------
            pend_av = []        # (pot, hp, et, ch): AV trails by two ticks
            deferred = []       # norm/proj actions, one per tick
            body_reps = REPS if (REPS > 1 and REPS_MODE == "unroll") else 1
            its = [
                (qb, hp)
                for _ in range(body_reps)
                for qb in range(QH // 512)
                for hp in range(H // 2)
            ]
            for qb, hp in its:
                pots = None
                for ch in range(NCH):
                    s = ps.tile([P, 1024], F32, tag="s", name="s")
                    for e in range(2):
                        h = 2 * hp + e
                        r, cc = 32 * (h % 4), h // 4
                        nc.tensor.matmul(
                            s[:, 512 * e : 512 * e + 512],
                            lhsT=kT_sb[r : r + 32, cc, 128 * ch : 128 * ch + 128],
                            rhs=qT_sb[r : r + 32, cc, 512 * qb : 512 * qb + 512],
                            start=True,
                            stop=True,
                            tile_position=(r, 0),
                        )
                    # exp straight from PSUM: ScalarE takes the first ACT_COLS
                    # columns; GPSIMD (pow) takes the rest via a DVE-staged,
                    # pre-scaled SBUF copy (GPSIMD has no PSUM port).
                    etw = epool.tile([P, 1024], F32R, tag="E", name="etw")
                    nc.scalar.activation(
                        etw[:, 0:ACT_COLS], s[:, 0:ACT_COLS], EXP, scale=SCALE
                    )
                    if POOL_COLS:
                        scp = spool.tile([P, POOL_COLS], F32, tag="SC", name="scp")
                        nc.vector.tensor_scalar_mul(scp[:], s[:, ACT_COLS:], SCALE)
                        nc.gpsimd.tensor_tensor(
                            etw[:, ACT_COLS:], econst[:], scp[:], mybir.AluOpType.pow
                        )
                    if pots is None:
                        pots = (
                            po.tile([D + 1, 512], F32, tag="o", name="pot0"),
                            po.tile([D + 1, 512], F32, tag="o", name="pot1"),
                        )
                    # AV trails its chunk by three ticks so the full exp
                    # pipeline latency (PSUM scores -> DVE stage -> GPSIMD pow
                    # -> E tile) is hidden and the PE never stalls on E.
                    pend_av.append((pots, hp, etw[:], ch))
                    if len(pend_av) > 3:
                        emit_av(*pend_av.pop(0))
                    if deferred:
                        deferred.pop(0)()
                    elif worklist:
                        worklist.pop(0)()
                if (qb, hp) != its[-1]:
                    # spacers: the first norm must emit after this iteration's
                    # chunk-15 AV, which flushes at tick 2 of the next
                    # iteration (3-deep AV queue).
                    deferred.append(lambda: None)
                    deferred.append(lambda: None)
                    box = {}
                    for e in range(2):
                        deferred.append(
                            lambda bx=box, a=pots, ee=e: bx.__setitem__(
                                ("r", ee), norm_rcp(a, ee)
                            )
                        )
                        deferred.append(
                            lambda bx=box, ee=e: bx.__setitem__(
                                ("b", ee), norm_bcast(bx[("r", ee)])
                            )
                        )
                        deferred.append(
                            lambda bx=box, a=pots, b=hp, c=qb, ee=e: norm_mul(
                                bx[("b", ee)], a, b, c, ee
                            )
                        )
                    if hp == H // 2 - 1:
                        for qt in range(4):
                            deferred.append(lambda a=qb, b=qt: emit_proj_qt(a, b))
            for pa in pend_av:
                emit_av(*pa)
            for act in deferred:
                act()
            # fast epilogue for the final head pair: interleave both heads'
            # chains and broadcast with plain-fp32 matmuls (PE is idle here),
            # skipping the fp32r rounding copy on the reciprocal.
            l_qb, l_hp = its[-1]
            rcpfs = []
            for e in range(2):
                rcpf = small.tile([1, 512], F32, tag="rcpf", name="rcpf")
                nc.vector.reciprocal(rcpf[:], pots[e][D : D + 1, :])
                rcpfs.append(rcpf)
            for e in range(2):
                bc = ps.tile([P, 1024], F32, tag="s", name="bcT")
                nc.tensor.matmul(
                    bc[0:32, 0:512], lhsT=ones_f2[:], rhs=rcpfs[e][:],
                    start=True, stop=True,
                )
                onr = small.tile([32, 512], F32, tag="onr", name="onr")
                nc.vector.tensor_copy(onr[:], pots[e][0:D, :])
                h = 2 * l_hp + e
                r, cc = 32 * (h % 4), h // 4
                nc.vector.tensor_mul(
                    OT_sb[r : r + 32, cc, 512 * l_qb : 512 * l_qb + 512],
                    onr[:],
                    bc[0:32, 0:512],
                )
            for qt in range(4):
                tq = 4 * l_qb + qt
                py = ps.tile([P, 1024], F32, tag="s", name="pyT")
                for ci in range(2):
                    nc.tensor.matmul(
                        py[:, 0:256],
                        lhsT=OT_sb[:, ci, 128 * tq : 128 * tq + 128],
                        rhs=pw_sb[:, ci, :],
                        start=(ci == 0),
                        stop=(ci == 1),
                    )
                ysb = ypool.tile([P, C], F32, tag="y", name="ysb")
                nc.vector.tensor_add(ysb[:], py[:, 0:256], pb_sb[:])
                nc.sync.dma_start(y[128 * tq : 128 * tq + 128, :], ysb[:])

        if REPS == 1 or REPS_MODE == "unroll":
            emit_body()
        else:
            with tc.For_i(0, REPS, 1):
                emit_body()


_NC = None
_RUNNER = None


def _get_runner():
    """Cached SPMD runner: builds the jitted shard_map executable once so warm
    kernel() calls skip JAX retracing/compilation (run_bass_kernel_spmd builds
    a fresh closure per call, which always misses the jit cache)."""
    global _RUNNER
    if _RUNNER is not None:
        return _RUNNER
    import jax
    from jax.sharding import Mesh, PartitionSpec
    from jax.experimental.shard_map import shard_map
    from concourse import bass2jax, mybir as _mb

    nc = _get_nc()
    bass2jax.install_neuronx_cc_hook()

    assert nc.dbg_addr is None
    partition_name = nc.partition_id_tensor.name if nc.partition_id_tensor else None
    in_names, out_names, out_avals = [], [], []
    for alloc in nc.m.functions[0].allocations:
        if not isinstance(alloc, _mb.MemoryLocationSet):
            continue
        name = alloc.memorylocations[0].name
        if alloc.kind == "ExternalInput":
            if name != partition_name:
                in_names.append(name)
        elif alloc.kind == "ExternalOutput":
            out_names.append(name)
            out_avals.append(
                jax.core.ShapedArray(tuple(alloc.tensor_shape), _mb.dt.np(alloc.dtype))
            )
    n_params = len(in_names)
    n_outs = len(out_avals)
    all_names = in_names + out_names
    if partition_name is not None:
        all_names = all_names + [partition_name]

    def _body(*args):
        operands = list(args)
        if partition_name is not None:
            operands.append(bass2jax.partition_id_tensor())
        outs = bass2jax._bass_exec_p.bind(
            *operands,
            out_avals=tuple(out_avals),
            in_names=tuple(all_names),
            out_names=tuple(out_names),
            lowering_input_output_aliases=(),
            sim_require_finite=True,
            sim_require_nnan=True,
            nc=nc,
        )
        return tuple(outs)

    devices = jax.devices()[:NCORES]
    mesh = Mesh(np.asarray(devices), ("core",))
    sharded = jax.jit(
        shard_map(
            _body,
            mesh=mesh,
            in_specs=(PartitionSpec("core"),) * (n_params + n_outs),
            out_specs=(PartitionSpec("core"),) * n_outs,
            check_rep=False,
        ),
        donate_argnums=tuple(range(n_params, n_params + n_outs)),
        keep_unused=True,
    )

    def run(in_maps):
        concat_in = [
            np.concatenate([np.asarray(m[nm]) for m in in_maps], axis=0)
            for nm in in_names
        ]
        concat_zeros = [
            np.zeros((NCORES * a.shape[0], *a.shape[1:]), a.dtype) for a in out_avals
        ]
        out_arrs = sharded(*concat_in, *concat_zeros)
        return [
            {
                nm: np.asarray(out_arrs[i]).reshape(NCORES, *out_avals[i].shape)[c]
                for i, nm in enumerate(out_names)
            }
            for c in range(NCORES)
        ]

    _RUNNER = run
    return run


def _get_nc():
    global _NC
    if _NC is None:
        nc = bacc.Bacc("TRN2", target_bir_lowering=False, debug=False, num_devices=1)
        xT = nc.dram_tensor("xT", [C, N], F32, kind="ExternalInput").ap()
        xTq = nc.dram_tensor("xTq", [C, QH], F32, kind="ExternalInput").ap()
        wall = nc.dram_tensor("wall", [C, 4 * C], F32, kind="ExternalInput").ap()
        pb = nc.dram_tensor("pb", [C], F32, kind="ExternalInput").ap()
        y = nc.dram_tensor("y", [QH, C], F32, kind="ExternalOutput").ap()
        with tile.TileContext(nc) as tc:
            _emit(tc, xT, xTq, wall, pb, y)
        nc.finalize()
        _NC = nc
    return _NC


def kernel(x, qkv_w, proj_w, proj_b):
    x = np.asarray(x, dtype=np.float32)
    qkv_w = np.asarray(qkv_w, dtype=np.float32)
    proj_w = np.asarray(proj_w, dtype=np.float32)
    proj_b = np.asarray(proj_b, dtype=np.float32)

    nc = _get_nc()
    wall = np.ascontiguousarray(
        np.stack(
            [qkv_w[C : 2 * C].T, qkv_w[0:C].T, qkv_w[2 * C : 3 * C].T, proj_w.T],
            axis=1,
        ).reshape(C, 4 * C)
    )

    in_maps = []
    for c in range(NCORES):
        b, qh = c // 2, c % 2
        xT = np.ascontiguousarray(x[b].T)
        in_maps.append(
            {
                "xT": xT,
                "xTq": np.ascontiguousarray(xT[:, qh * QH : (qh + 1) * QH]),
                "wall": wall,
                "pb": proj_b,
            }
        )
    if os.environ.get("BASS_ATTN_TIMING"):
        import time as _t

        t0 = _t.time()
        run = _get_runner()
        t1 = _t.time()
        results = run(in_maps)
        t2 = _t.time()
        out = np.empty((B, N, C), np.float32)
        for c in range(NCORES):
            b, qh = c // 2, c % 2
            out[b, qh * QH : (qh + 1) * QH] = results[c]["y"]
        t3 = _t.time()
        print(
            f"[timing] runner={t1-t0:.3f}s exec={t2-t1:.3f}s gather={t3-t2:.3f}s",
            flush=True,
        )
        return out
    results = _get_runner()(in_maps)
    out = np.empty((B, N, C), np.float32)
    for c in range(NCORES):
        b, qh = c // 2, c % 2
        out[b, qh * QH : (qh + 1) * QH] = results[c]["y"]
    return out

